# revision 7
# baseline (speedup 1.0000x reference)
"""Multi-label masked-gather mean loss on 8 Trainium2 NeuronCores.

reference:
    logp = log_softmax(x, -1); per_sample = -sum_t(mask*logp[i, y[i,t]])/count_i
    loss = mean(per_sample)

Identity used (count_i > 0):
    per_sample_i = logsumexp(x_i) - sum_t w[i,t] * x[i, y[i,t]],  w = mask/count
    loss = (sum_i logsumexp(x_i) + sum_{i,t} wneg[i,t] * x[i,y[i,t]]) / B
with wneg = -w. Data-parallel over the batch: 4096 rows -> 512 rows/core.

Per core the Bass kernel streams its x shard [512, 50257] f32 once from HBM
(memory-bound), computing exp + row-sum via ScalarE activation accumulate,
logsumexp per row, an indirect-DMA gather of the 8 labeled logits per row,
and reduces everything to a per-partition partial sum [128, 1].
Host sums the 8x128 partials and divides by B.
"""

import sys

sys.path.insert(0, "/opt/trn_rl_repo")

import math

import numpy as np

import concourse.bass as bass
import concourse.tile as tile
from concourse import bacc, mybir
from concourse import bass_utils

# Problem shape (hardcoded per contract)
B, C, T = 4096, 50257, 8
NCORES = 8
BL = B // NCORES  # 512 rows per core
P = 128
RB = BL // P      # 4 row blocks per core
CW = 8192         # column tile width (f32 -> 32 KiB per partition)
NCT = math.ceil(C / CW)  # 7 column tiles
GCOLS = BL * T // P      # 32: gathered elements per partition

_f32 = mybir.dt.float32
_i32 = mybir.dt.int32

_compiled = None  # (nc, names) cache


def _build():
    nc = bacc.Bacc(
        "TRN2",
        target_bir_lowering=False,
        debug=False,
        enable_asserts=False,
        num_devices=NCORES,
    )
    x_t = nc.dram_tensor("x", [BL, C], _f32, kind="ExternalInput")
    idx_t = nc.dram_tensor("idx", [P, GCOLS], _i32, kind="ExternalInput")
    wneg_t = nc.dram_tensor("wneg", [P, GCOLS], _f32, kind="ExternalInput")
    # cols 0..RB-1: per-row sumexp per row block; col RB: sum_t wneg*gathered
    out_t = nc.dram_tensor("out", [P, RB + 1], _f32, kind="ExternalOutput")

    x = x_t.ap()
    idx = idx_t.ap()
    wneg = wneg_t.ap()
    out = out_t.ap()

    with tile.TileContext(nc) as tc:
        with (
            tc.tile_pool(name="xin", bufs=5) as xin_pool,
            tc.tile_pool(name="scratch", bufs=1) as scratch_pool,
            tc.tile_pool(name="stats", bufs=1) as stats_pool,
            tc.tile_pool(name="gather", bufs=1) as gather_pool,
        ):
            # exp output scratch: values are unused, only accum_out matters.
            exp_scratch = scratch_pool.tile([P, CW], _f32)
            # per (row-block, col-tile) partial sumexp
            acc = stats_pool.tile([P, RB * NCT], _f32)
            # result: cols 0..RB-1 = per-row sumexp; col RB = gather dot
            res = stats_pool.tile([P, RB + 1], _f32)

            # --- gather path (tiny; overlaps the big stream; SWDGE only
            # so the Sync HWDGE ring carries nothing but the stream) ---
            idx_tile = gather_pool.tile([P, GCOLS], _i32)
            nc.gpsimd.dma_start(out=idx_tile[:], in_=idx[:])
            w_tile = gather_pool.tile([P, GCOLS], _f32)
            nc.gpsimd.dma_start(out=w_tile[:], in_=wneg[:])
            g_tile = gather_pool.tile([P, GCOLS], _f32)
            nc.gpsimd.indirect_dma_start(
                out=g_tile[:],
                out_offset=None,
                in_=x[:],
                in_offset=bass.IndirectOffsetOnAxis(ap=idx_tile[:], axis=1),
            )
            gw = gather_pool.tile([P, GCOLS], _f32)
            nc.vector.tensor_tensor(
                out=gw[:], in0=g_tile[:], in1=w_tile[:], op=mybir.AluOpType.mult
            )
            nc.vector.tensor_reduce(
                out=res[:, RB : RB + 1],
                in_=gw[:],
                axis=mybir.AxisListType.X,
                op=mybir.AluOpType.add,
            )

            # --- main stream: exp + row-sum of x shard ---
            # column-major order: the four small remainder tiles land last,
            # so ACT's post-stream serial tail is short.
            for j in range(NCT):
                c0 = j * CW
                cw = min(CW, C - c0)
                for rb in range(RB):
                    xt = xin_pool.tile([P, CW], _f32, tag="xt")
                    nc.sync.dma_start(
                        out=xt[:, :cw], in_=x[rb * P : (rb + 1) * P, c0 : c0 + cw]
                    )
                    nc.scalar.activation(
                        out=exp_scratch[:, :cw],
                        in_=xt[:, :cw],
                        func=mybir.ActivationFunctionType.Exp,
                        accum_out=acc[:, rb * NCT + j : rb * NCT + j + 1],
                    )
            for rb in range(RB):
                nc.vector.tensor_reduce(
                    out=res[:, rb : rb + 1],
                    in_=acc[:, rb * NCT : (rb + 1) * NCT],
                    axis=mybir.AxisListType.X,
                    op=mybir.AluOpType.add,
                )
            nc.sync.dma_start(out=out[:], in_=res[:])

    nc.compile()
    return nc


def _get_compiled():
    global _compiled
    if _compiled is None:
        _compiled = _build()
    return _compiled


def _make_in_maps(x, y):
    x = np.ascontiguousarray(np.asarray(x, dtype=np.float32))
    y = np.asarray(y)
    mask = y != -1
    cnt = mask.sum(axis=1)
    # rows with count 0 would be NaN in the reference; inputs never hit this
    w = np.where(mask, 1.0 / np.maximum(cnt, 1)[:, None], 0.0).astype(np.float32)
    wneg = -w
    safe = np.where(mask, y, 0).astype(np.int64)

    in_maps = []
    for m in range(NCORES):
        sl = slice(m * BL, (m + 1) * BL)
        xs = x[sl]
        flat = (
            np.arange(BL, dtype=np.int64)[:, None] * C + safe[sl]
        ).astype(np.int32)
        in_maps.append(
            {
                "x": xs,
                "idx": np.ascontiguousarray(flat.reshape(P, GCOLS)),
                "wneg": np.ascontiguousarray(wneg[sl].reshape(P, GCOLS)),
            }
        )
    return in_maps


def kernel(**inputs) -> np.ndarray:
    x, y = inputs["x"], inputs["y"]
    nc = _get_compiled()
    in_maps = _make_in_maps(x, y)
    res = bass_utils.run_bass_kernel_spmd(
        nc, in_maps, core_ids=list(range(NCORES))
    )
    total = 0.0
    for r in res.results:
        out = np.asarray(r["out"], dtype=np.float64)  # [P, RB+1]
        total += np.log(out[:, :RB]).sum() + out[:, RB].sum()
    return np.float32(total / B)


# revision 8
# speedup vs baseline: 1.2262x; 1.2262x over previous
"""Multi-label masked-gather mean loss on 8 Trainium2 NeuronCores.

reference:
    logp = log_softmax(x, -1); per_sample = -sum_t(mask*logp[i, y[i,t]])/count_i
    loss = mean(per_sample)

Identity used (count_i > 0):
    per_sample_i = logsumexp(x_i) - sum_t w[i,t] * x[i, y[i,t]],  w = mask/count
    loss = (sum_i logsumexp(x_i) + sum_{i,t} wneg[i,t] * x[i,y[i,t]]) / B
with wneg = -w. Data-parallel over the batch: 4096 rows -> 512 rows/core.

Per core the Bass kernel streams its x shard [512, 50257] f32 once from HBM
(memory-bound), computing exp + row-sum via ScalarE activation accumulate,
logsumexp per row, an indirect-DMA gather of the 8 labeled logits per row,
and reduces everything to a per-partition partial sum [128, 1].
Host sums the 8x128 partials and divides by B.
"""

import sys

sys.path.insert(0, "/opt/trn_rl_repo")

import math

import numpy as np

import concourse.bass as bass
import concourse.tile as tile
from concourse import bacc, mybir
from concourse import bass_utils

# Problem shape (hardcoded per contract)
B, C, T = 4096, 50257, 8
NCORES = 8
BL = B // NCORES  # 512 rows per core
P = 128
RB = BL // P      # 4 row blocks per core
CW = 8192         # column tile width (f32 -> 32 KiB per partition)
NCT = math.ceil(C / CW)  # 7 column tiles
GCOLS = BL * T // P      # 32: gathered elements per partition

_f32 = mybir.dt.float32
_i32 = mybir.dt.int32

_compiled = None  # (nc, names) cache


def _build():
    nc = bacc.Bacc(
        "TRN2",
        target_bir_lowering=False,
        debug=False,
        enable_asserts=False,
        num_devices=NCORES,
    )
    x_t = nc.dram_tensor("x", [BL, C], _f32, kind="ExternalInput")
    idx_t = nc.dram_tensor("idx", [P, GCOLS], _i32, kind="ExternalInput")
    wneg_t = nc.dram_tensor("wneg", [P, GCOLS], _f32, kind="ExternalInput")
    # cols 0..RB-1: per-row sumexp per row block; col RB: sum_t wneg*gathered
    out_t = nc.dram_tensor("out", [P, RB + 1], _f32, kind="ExternalOutput")

    x = x_t.ap()
    idx = idx_t.ap()
    wneg = wneg_t.ap()
    out = out_t.ap()

    with tile.TileContext(nc) as tc:
        with (
            tc.tile_pool(name="xin", bufs=5) as xin_pool,
            tc.tile_pool(name="scratch", bufs=1) as scratch_pool,
            tc.tile_pool(name="stats", bufs=1) as stats_pool,
            tc.tile_pool(name="gather", bufs=1) as gather_pool,
        ):
            # exp output scratch: values are unused, only accum_out matters.
            exp_scratch = scratch_pool.tile([P, CW], _f32)
            # per (row-block, col-tile) partial sumexp
            acc = stats_pool.tile([P, RB * NCT], _f32)
            # result: cols 0..RB-1 = per-row sumexp; col RB = gather dot
            res = stats_pool.tile([P, RB + 1], _f32)

            # --- gather path (tiny; overlaps the big stream; SWDGE only
            # so the Sync HWDGE ring carries nothing but the stream) ---
            idx_tile = gather_pool.tile([P, GCOLS], _i32)
            nc.gpsimd.dma_start(out=idx_tile[:], in_=idx[:])
            w_tile = gather_pool.tile([P, GCOLS], _f32)
            nc.gpsimd.dma_start(out=w_tile[:], in_=wneg[:])
            g_tile = gather_pool.tile([P, GCOLS], _f32)
            nc.gpsimd.indirect_dma_start(
                out=g_tile[:],
                out_offset=None,
                in_=x[:],
                in_offset=bass.IndirectOffsetOnAxis(ap=idx_tile[:], axis=1),
            )
            gw = gather_pool.tile([P, GCOLS], _f32)
            nc.vector.tensor_tensor(
                out=gw[:], in0=g_tile[:], in1=w_tile[:], op=mybir.AluOpType.mult
            )
            nc.vector.tensor_reduce(
                out=res[:, RB : RB + 1],
                in_=gw[:],
                axis=mybir.AxisListType.X,
                op=mybir.AluOpType.add,
            )

            # --- main stream: exp + row-sum of x shard ---
            for rb in range(RB):
                for j in range(NCT):
                    c0 = j * CW
                    cw = min(CW, C - c0)
                    xt = xin_pool.tile([P, CW], _f32, tag="xt")
                    nc.sync.dma_start(
                        out=xt[:, :cw], in_=x[rb * P : (rb + 1) * P, c0 : c0 + cw]
                    )
                    nc.scalar.activation(
                        out=exp_scratch[:, :cw],
                        in_=xt[:, :cw],
                        func=mybir.ActivationFunctionType.Exp,
                        accum_out=acc[:, rb * NCT + j : rb * NCT + j + 1],
                    )
                nc.vector.tensor_reduce(
                    out=res[:, rb : rb + 1],
                    in_=acc[:, rb * NCT : (rb + 1) * NCT],
                    axis=mybir.AxisListType.X,
                    op=mybir.AluOpType.add,
                )
            nc.sync.dma_start(out=out[:], in_=res[:])

    nc.compile()
    return nc


def _get_compiled():
    global _compiled
    if _compiled is None:
        _compiled = _build()
    return _compiled


def _make_in_maps(x, y):
    x = np.ascontiguousarray(np.asarray(x, dtype=np.float32))
    y = np.asarray(y)
    mask = y != -1
    cnt = mask.sum(axis=1)
    # rows with count 0 would be NaN in the reference; inputs never hit this
    w = np.where(mask, 1.0 / np.maximum(cnt, 1)[:, None], 0.0).astype(np.float32)
    wneg = -w
    safe = np.where(mask, y, 0).astype(np.int64)

    in_maps = []
    for m in range(NCORES):
        sl = slice(m * BL, (m + 1) * BL)
        xs = x[sl]
        flat = (
            np.arange(BL, dtype=np.int64)[:, None] * C + safe[sl]
        ).astype(np.int32)
        in_maps.append(
            {
                "x": xs,
                "idx": np.ascontiguousarray(flat.reshape(P, GCOLS)),
                "wneg": np.ascontiguousarray(wneg[sl].reshape(P, GCOLS)),
            }
        )
    return in_maps


def kernel(**inputs) -> np.ndarray:
    x, y = inputs["x"], inputs["y"]
    nc = _get_compiled()
    in_maps = _make_in_maps(x, y)
    res = bass_utils.run_bass_kernel_spmd(
        nc, in_maps, core_ids=list(range(NCORES))
    )
    total = 0.0
    for r in res.results:
        out = np.asarray(r["out"], dtype=np.float64)  # [P, RB+1]
        total += np.log(out[:, :RB]).sum() + out[:, RB].sum()
    return np.float32(total / B)
